# revision 38
# baseline (speedup 1.0000x reference)
"""Trainium2 Bass kernel for nn_Attention (LN -> QKV -> MHA -> out-proj).

Sharding: 8 cores = 4 batches x 2 head-groups. Core c handles batch c//2,
heads (c%2)*8 .. (c%2)*8+8 (tensor-parallel split of w_qkv columns / w_out
rows). Each core emits a partial [2048, 1024] output; host sums the two
partials per batch.

Device algorithm (per core), all matmuls bf16 with f32 PSUM accumulation:
  1. LayerNorm x (f32 stats via bn_stats), cast bf16, PE-transpose to
     xnT [1024, 2048] (features on partitions), fusing gamma/beta into the
     PSUM->SBUF copy. Weight DMAs are issued first so the PE never waits.
  2. qT/kT [512, 2048] = wq/wk^T @ xnT; v [2048, 512] natural layout with an
     appended ones column (denominator trick).
  3. Per head pair (2j at partitions 0:64, 2j+1 at 64:128): scoresT
     [kv, q] = kz^T qT with k stored zero-padded to all 128 contraction
     partitions (stale weights in unused PE rows burn full dynamic power;
     zeros keep them quiet); e = exp(scale * scoresT) (no max subtraction
     -- scores are O(5) for these inputs); PV with ones-augmented v gives
     [out_unnorm^T; denom] in PSUM. q is processed in four 512-token
     windows so each PE burst (~28us) stays under the power governor's
     ~48us sustained-activity clamp trigger, with a forced normalize
     bubble between windows (single-buffered PV accumulators) for credit
     recovery -- this schedule shape is worth ~20% end-to-end vs a dense
     one. The next pair's qkv matmul groups interleave into the kv loop.
     Normalization: DVE reciprocal of the denom row, ACT bf16 convert,
     K=1 ones-matmul broadcast, one full-width multiply.
  4. out = sum_j outn[j]^T @ w_out[j], K=128 PSUM accumulation over the 4
     head-pair blocks.
"""

import os
import sys

sys.path.insert(0, "/opt/trn_rl_repo")
os.environ.setdefault("MYCRO_LOCAL_CACHE", "1")

import numpy as np
import ml_dtypes

N_TOK = 2048
DIM = 1024
HPC = 8          # heads per core
DH = 64          # head dim
INNER_C = HPC * DH  # 512 per-core inner width
NT = N_TOK // 128   # 16 token tiles
KF = DIM // 128     # 8 feature tiles
SCALE = DH ** -0.5

_cache = {}


def _build_nc():
    import concourse.bass as bass
    import concourse.mybir as mybir
    import concourse.tile as tile
    from concourse import bacc
    from concourse.masks import make_identity
    from contextlib import ExitStack

    f32 = mybir.dt.float32
    bf16 = mybir.dt.bfloat16
    AF = mybir.ActivationFunctionType
    nc = bacc.Bacc(None, target_bir_lowering=False)

    x_d = nc.dram_tensor("x", [N_TOK, DIM], f32, kind="ExternalInput")
    gamma_d = nc.dram_tensor("gamma", [DIM], f32, kind="ExternalInput")
    beta_d = nc.dram_tensor("beta", [DIM], f32, kind="ExternalInput")
    wqkv_d = nc.dram_tensor("wqkv", [DIM, 3 * INNER_C], bf16, kind="ExternalInput")
    wout_d = nc.dram_tensor("wout", [INNER_C, DIM], bf16, kind="ExternalInput")
    out_d = nc.dram_tensor("out", [N_TOK, DIM], f32, kind="ExternalOutput")

    with tile.TileContext(nc) as tc, ExitStack() as ctx:
        consts = ctx.enter_context(tc.tile_pool(name="consts", bufs=1))
        weights = ctx.enter_context(tc.tile_pool(name="weights", bufs=1))
        persist = ctx.enter_context(tc.tile_pool(name="persist", bufs=1))
        work = ctx.enter_context(tc.tile_pool(name="work", bufs=3))
        nrm = ctx.enter_context(tc.tile_pool(name="nrm", bufs=1))
        stats = ctx.enter_context(tc.tile_pool(name="stats", bufs=4))
        mm_ps = ctx.enter_context(tc.tile_pool(name="mm_ps", bufs=3, space="PSUM"))
        pv_ps = ctx.enter_context(tc.tile_pool(name="pv_ps", bufs=2, space="PSUM"))

        ident = consts.tile([128, 128], bf16, tag="ident")
        make_identity(nc, ident)
        eps_t = consts.tile([128, 1], f32, tag="eps")
        nc.vector.memset(eps_t, 1e-5)
        gcols = consts.tile([128, KF], f32, tag="gcols")
        nc.sync.dma_start(out=gcols, in_=gamma_d.rearrange("(f p) -> p f", p=128))
        bcols = consts.tile([128, KF], f32, tag="bcols")
        ones_t = consts.tile([1, 64], bf16, tag="ones_t")
        nc.vector.memset(ones_t, 1.0)
        nc.sync.dma_start(out=bcols, in_=beta_d.rearrange("(f p) -> p f", p=128))

        wo_sb = [weights.tile([128, DIM], bf16, tag=f"wo{j}", name=f"wo{j}")
                 for j in range(4)]

        qkT = [persist.tile([128, N_TOK], bf16, tag=f"qkT{m}", name=f"qkT{m}") for m in range(4)]
        # k tiles stored zero-padded to the full 128 contraction partitions:
        # kz[j][r] has head 2j+r's 64 dims at partition offset r*64 and exact
        # zeros elsewhere. The K=64 score matmuls otherwise leave stale
        # weights in the unused 64 PE rows multiplying the q stream at full
        # dynamic power; zero weights keep those rows quiet, and PE power is
        # what the activity governor clamps on.
        kz = [[persist.tile([128, N_TOK], bf16, tag=f"kz{j}{r}", name=f"kz{j}{r}")
               for r in range(2)] for j in range(4)]
        outn = [persist.tile([128, N_TOK], bf16, tag=f"outn{j}", name=f"outn{j}") for j in range(4)]
        v_aug = [persist.tile([128, HPC, DH + 1], bf16, tag=f"vaug{t}", name=f"vaug{t}")
                 for t in range(NT)]
        for j in range(4):
            nc.vector.memset(kz[j][0][64:128, :], 0.0)
            nc.vector.memset(kz[j][1][0:64, :], 0.0)

        with tc.tile_pool(name="qkvw", bufs=1) as qkvw:
            wq_sb = [qkvw.tile([128, 3 * INNER_C], bf16, tag=f"wq{kc}",
                               name=f"wq{kc}") for kc in range(KF)]
            xnT = [qkvw.tile([128, N_TOK], bf16, tag=f"xnT{f}", name=f"xnT{f}") for f in range(KF)]

            def v_group(t):
                vt = v_aug[t]
                nc.vector.memset(vt[:, :, DH:DH + 1], 1.0)
                ps = mm_ps.tile([128, 1024], f32, tag="mm", name="vps")
                for kc in range(KF):
                    nc.tensor.matmul(
                        ps[:, 0:512],
                        lhsT=xnT[kc][:, t * 128:(t + 1) * 128],
                        rhs=wq_sb[kc][:, 2 * INNER_C:3 * INNER_C],
                        start=(kc == 0), stop=(kc == KF - 1),
                    )
                nc.scalar.copy(
                    out=vt[:, :, 0:DH],
                    in_=ps[:, 0:512].rearrange("p (h d) -> p h d", h=HPC),
                )

            def qk_group(m, n):
                ps = mm_ps.tile([128, 1024], f32, tag="mm", name="qkps")
                for kc in range(KF):
                    nc.tensor.matmul(
                        ps[:, 0:512],
                        lhsT=wq_sb[kc][:, m * 128:(m + 1) * 128],
                        rhs=xnT[kc][:, n * 512:(n + 1) * 512],
                        start=(kc == 0), stop=(kc == KF - 1),
                    )
                ns = slice(n * 512, (n + 1) * 512)
                if m < 4:
                    nc.vector.tensor_copy(out=qkT[m][:, ns], in_=ps[:, 0:512])
                else:
                    # psum partitions 0:64 are head 2j's k dims, 64:128 are
                    # head 2j+1's -- scatter into the zero-padded kz tiles
                    j = m - 4
                    nc.vector.tensor_copy(out=kz[j][0][0:64, ns],
                                          in_=ps[0:64, 0:512])
                    nc.vector.tensor_copy(out=kz[j][1][64:128, ns],
                                          in_=ps[64:128, 0:512])

            # ------------ Phase 1: LayerNorm + transpose ------------
            with tc.tile_pool(name="ln", bufs=1) as lnp:
                for g in range(4):
                    xn_grp = []
                    for tt in range(4):
                        t = g * 4 + tt
                        xt = work.tile([128, DIM], f32, tag="xt", bufs=3)
                        nc.sync.dma_start(out=xt,
                                          in_=x_d[t * 128:(t + 1) * 128, :])
                        st = stats.tile([128, 2, 6], f32, tag="bn")
                        xr = xt.rearrange("p (s d) -> p s d", s=2)
                        for s in range(2):
                            nc.vector.bn_stats(out=st[:, s, :], in_=xr[:, s, :])
                        mv = stats.tile([128, 2], f32, tag="mv")
                        nc.vector.bn_aggr(out=mv, in_=st)
                        rsig = stats.tile([128, 1], f32, tag="rsig")
                        nc.scalar.activation(
                            out=rsig, in_=mv[:, 1:2],
                            func=AF.Sqrt,
                            bias=eps_t, scale=1.0,
                        )
                        nc.vector.reciprocal(out=rsig, in_=rsig)
                        xn = lnp.tile([128, DIM], bf16, tag=f"xn{tt}",
                                      name=f"xn{tt}", bufs=2)
                        nc.vector.tensor_scalar(
                            out=xn, in0=xt, scalar1=mv[:, 0:1], scalar2=rsig,
                            op0=mybir.AluOpType.subtract, op1=mybir.AluOpType.mult,
                        )
                        xn_grp.append(xn)
                    if g == 0:
                        for kc in range(KF):
                            # Activation HWDGE queue: wq streams in
                            # parallel with the x tiles on the SP queue
                            # instead of behind them (the PE otherwise
                            # idles ~23us before the first v/qk matmuls)
                            nc.scalar.dma_start(
                                out=wq_sb[kc],
                                in_=wqkv_d[kc * 128:(kc + 1) * 128, :])

                    for f in range(KF):
                        ps = mm_ps.tile([128, 512], bf16, tag="mm", name="trps")
                        for tt in range(4):
                            nc.tensor.transpose(
                                out=ps[:, tt * 128:(tt + 1) * 128],
                                in_=xn_grp[tt][:, f * 128:(f + 1) * 128],
                                identity=ident,
                            )
                        nc.scalar.activation(
                            out=xnT[f][:, g * 512:(g + 1) * 512],
                            in_=ps[:, 0:512],
                            func=AF.Identity,
                            scale=gcols[:, f:f + 1], bias=bcols[:, f:f + 1],
                        )
                    for tt in range(4):
                        v_group(g * 4 + tt)
                    qk_group(0, g)
                    qk_group(4, g)

            def attention_pair(j, bg=()):
                bg = list(bg)
                # Heads 2j (partitions 0:64) and 2j+1 (64:128) run
                # concurrently: their K=64 score matmuls go to disjoint PE
                # row-groups and separate PSUM banks. q is split in four
                # 512-token windows: each burst (~28us) stays under the
                # power governor's ~48us sustained-activity trigger, and
                # the single-buffered PV accumulators force a normalize
                # bubble between windows for credit recovery.
                for qp in range(4):
                    qlo = qp * 512
                    ps_pv = [pv_ps.tile([65, 512], f32, tag=f"pv{r}",
                                        name="pspv", bufs=1) for r in range(2)]
                    for t in range(NT):
                        if t % 4 == 0 and bg:
                            bg.pop(0)()
                        ets = []
                        for r in range(2):
                            # full-width rhs (both heads' q dims); the
                            # complementary 64 partitions hit zero weights
                            ps_s = mm_ps.tile([128, 1024], f32, tag="mm")
                            nc.tensor.matmul(
                                ps_s[:, 0:512],
                                lhsT=kz[j][r][:, t * 128:(t + 1) * 128],
                                rhs=qkT[j][:, qlo:qlo + 512],
                                start=True, stop=True,
                            )
                            et = work.tile([128, 1024], bf16, tag="et", bufs=4)
                            nc.scalar.activation(
                                out=et[:, 0:512], in_=ps_s[:, 0:512],
                                func=AF.Exp,
                                scale=SCALE,
                            )
                            ets.append(et)
                        for r in range(2):
                            nc.tensor.matmul(
                                ps_pv[r],
                                lhsT=v_aug[t][:, 2 * j + r, :],
                                rhs=ets[r][:, 0:512],
                                start=(t == 0), stop=(t == NT - 1),
                            )
                    for r in range(2):
                        rcf = nrm.tile([1, 512], f32, tag="rcf", bufs=4)
                        nc.vector.reciprocal(
                            out=rcf, in_=ps_pv[r][64:65, :])
                        rcp = nrm.tile([1, 512], bf16, tag="rcp", bufs=4)
                        nc.scalar.copy(out=rcp, in_=rcf)
                        rb = mm_ps.tile([64, 512], f32, tag="mm", name="rb")
                        nc.tensor.matmul(rb, lhsT=ones_t, rhs=rcp,
                                         start=True, stop=True)
                        un = nrm.tile([64, 512], bf16, tag="un", bufs=4)
                        nc.scalar.copy(out=un, in_=ps_pv[r][0:64, :])
                        nc.vector.tensor_mul(
                            out=outn[j][r * 64:(r + 1) * 64,
                                        qlo:qlo + 512],
                            in0=un, in1=rb,
                        )

            # out-proj weights: needed from pair 3 on; issue now so the
            # DMA never competes with x/wq traffic during phase 1
            for j in range(4):
                nc.sync.dma_start(out=wo_sb[j],
                                  in_=wout_d[j * 128:(j + 1) * 128, :])

            def op_tile(t):
                ps_o = mm_ps.tile([128, 1024], f32, tag="mm", name="ops")
                for c in range(2):
                    for jj in range(4):
                        nc.tensor.matmul(
                            ps_o[:, c * 512:(c + 1) * 512],
                            lhsT=outn[jj][:, t * 128:(t + 1) * 128],
                            rhs=wo_sb[jj][:, c * 512:(c + 1) * 512],
                            start=(jj == 0), stop=(jj == 3),
                        )
                osb = work.tile([128, DIM], f32, tag="osb", bufs=2)
                nc.scalar.copy(out=osb, in_=ps_o)
                nc.sync.dma_start(out=out_d[t * 128:(t + 1) * 128, :],
                                  in_=osb)

            for j in range(3):
                nxt = [(j + 1, n) for n in range(4)] + [(4 + j + 1, n) for n in range(4)]
                attention_pair(j, bg=[(lambda m=m, n=n: qk_group(m, n))
                                      for (m, n) in nxt])
            # pair 3 has no qk work left; interleave out-proj tiles
            # instead. Window qp writes outn columns for token tiles
            # 4qp..4qp+3, so window qp+1 can fold those tiles in -- the
            # first window pops no-ops (emitting an out-proj read before
            # its outn write would silently consume stale data). Pair 3's
            # bursts thereby match pairs 0-2 (~37us, still sub-trigger)
            # and the exposed epilogue shrinks to four tiles.
            nops = [(lambda: None) for _ in range(4)]
            ops = [(lambda t=t: op_tile(t)) for t in range(12)]
            attention_pair(3, bg=nops + ops)

        # ------------ Phase 4: output projection epilogue ------------
        for t in range(12, NT):
            op_tile(t)

    nc.compile()
    return nc


def get_nc():
    if "nc" not in _cache:
        _cache["nc"] = _build_nc()
    return _cache["nc"]


def shard_inputs(x, ln_gamma, ln_beta, w_qkv, w_out):
    """Returns per-core input maps (8 cores)."""
    bf = ml_dtypes.bfloat16
    x = np.asarray(x, np.float32)
    g = np.ascontiguousarray(np.asarray(ln_gamma, np.float32))
    b = np.ascontiguousarray(np.asarray(ln_beta, np.float32))
    w_qkv = np.asarray(w_qkv, np.float32)
    w_out = np.asarray(w_out, np.float32)
    in_maps = []
    for c in range(8):
        bi, gi = c // 2, c % 2
        wq = np.concatenate(
            [w_qkv[:, d * DIM + gi * INNER_C: d * DIM + (gi + 1) * INNER_C]
             for d in range(3)], axis=1).astype(bf)
        wo = np.ascontiguousarray(
            w_out[gi * INNER_C:(gi + 1) * INNER_C, :]).astype(bf)
        in_maps.append({
            "x": np.ascontiguousarray(x[bi]),
            "gamma": g,
            "beta": b,
            "wqkv": np.ascontiguousarray(wq),
            "wout": wo,
        })
    return in_maps


def gather_outputs(results):
    out = np.empty((4, N_TOK, DIM), np.float32)
    for bi in range(4):
        out[bi] = results[2 * bi]["out"] + results[2 * bi + 1]["out"]
    return out


def kernel(x, ln_gamma, ln_beta, w_qkv, w_out, **kw):
    from concourse.bass_utils import run_bass_kernel_spmd

    nc = get_nc()
    in_maps = shard_inputs(x, ln_gamma, ln_beta, w_qkv, w_out)
    res = run_bass_kernel_spmd(nc, in_maps, list(range(8)), **kw)
    _cache["last_results"] = res
    return gather_outputs(res.results)
